# revision 2
# baseline (speedup 1.0000x reference)
"""TRN2 Bass kernel for nn_Encoder_27290222198965.

Reference computation (N=8, L=2048, H=1024):
    q = x@Wq.T+bq ; k = x@Wk.T+bk ; v = x@Wv.T+bv
    d[n,l] = sum_h q*k                       (diagonal "attention" scores)
    att = softmax(diag-embed(d), axis=2) ->  colsum[n,l] = S[n] + (e-1)/(L-1+e),
        e = exp(d[n,l]), S[n] = sum_l 1/(L-1+exp(d[n,l]))
    out = (colsum[:, :, None] * v) @ Wo.T + bo

Algebraic refactor used here (validated to 4e-6 rel err in fp32):
    d[n,l]  = x_l^T (Wq^T Wk) x_l + (Wk^T bq + Wq^T bk)·x_l + bq·bk
            = rowsum(x ⊙ y') + c0,   y' = x @ M^T + u
    out     = colsum ⊙ (x @ Wc^T + bc) + bo,  Wc = Wo@Wv, bc = Wo@bv
so only TWO HxH projections run on hardware (y' and z) instead of four.

Sharding: data-parallel over N — core n handles batch n. All matmuls in
float32r (full PE rate at free dim >= 256, ~e8m12 effective precision).
Everything on-chip is kept transposed ([feature, token]) so all biases are
per-partition and fold into ScalarE psum->sbuf copies.
"""

import numpy as np

import concourse.bass as bass  # noqa: F401  (registers engines on Bacc)
import concourse.tile as tile
from concourse import bacc, mybir
from concourse.bass_utils import run_bass_kernel_spmd

dt = mybir.dt
AF = mybir.ActivationFunctionType
ALU = mybir.AluOpType

N, L, H = 8, 2048, 1024
P = 128            # SBUF partitions
LB = 512           # l-block (moving free dim of every matmul)
NH = H // P        # 8 h-blocks
NL = L // LB       # 4 l-blocks
N_CORES = 8

_CACHE = {}


def _build():
    nc = bacc.Bacc("TRN2", target_bir_lowering=False, debug=False,
                   num_devices=N_CORES)

    xT_d = nc.dram_tensor("xT", [H, L], dt.float32r, kind="ExternalInput").ap()
    MT_d = nc.dram_tensor("MT", [H, H], dt.float32r, kind="ExternalInput").ap()
    WcT_d = nc.dram_tensor("WcT", [H, H], dt.float32r, kind="ExternalInput").ap()
    ub_d = nc.dram_tensor("ub", [P, NH], dt.float32, kind="ExternalInput").ap()
    bcb_d = nc.dram_tensor("bcb", [P, NH], dt.float32, kind="ExternalInput").ap()
    bob_d = nc.dram_tensor("bob", [P, NH], dt.float32, kind="ExternalInput").ap()
    c0b_d = nc.dram_tensor("c0b", [P, 1], dt.float32, kind="ExternalInput").ap()
    ones_d = nc.dram_tensor("ones", [P, P], dt.float32r, kind="ExternalInput").ap()
    out_d = nc.dram_tensor("outT", [H, L], dt.float32, kind="ExternalOutput").ap()

    xT3 = xT_d.rearrange("(j p) l -> p j l", p=P)   # [128, 8, 2048]

    with tile.TileContext(nc) as tc:
        with (
            tc.tile_pool(name="resident", bufs=1) as rp,
            tc.tile_pool(name="xstream", bufs=2) as xp,
            tc.tile_pool(name="work", bufs=3) as wp,
            tc.tile_pool(name="mmpsum", bufs=3, space="PSUM") as yp,
            tc.tile_pool(name="dpsum", bufs=2, space="PSUM") as dp,
        ):
            # ---- resident loads ----
            mt = rp.tile([P, NH * H], dt.float32r)       # M^T  [hin, hb*1024+hout]
            wct = rp.tile([P, NH * H], dt.float32r)      # Wc^T
            for hb in range(NH):
                nc.sync.dma_start(mt[:, hb * H:(hb + 1) * H],
                                  MT_d[hb * P:(hb + 1) * P, :])
            ub = rp.tile([P, NH], dt.float32)
            nc.sync.dma_start(ub[:], ub_d[:])
            bcb = rp.tile([P, NH], dt.float32)
            nc.sync.dma_start(bcb[:], bcb_d[:])
            bob = rp.tile([P, NH], dt.float32)
            nc.sync.dma_start(bob[:], bob_d[:])
            c0b = rp.tile([P, 1], dt.float32)
            nc.sync.dma_start(c0b[:], c0b_d[:])
            ones = rp.tile([P, P], dt.float32r)
            nc.sync.dma_start(ones[:], ones_d[:])
            for hb in range(NH):
                nc.sync.dma_start(wct[:, hb * H:(hb + 1) * H],
                                  WcT_d[hb * P:(hb + 1) * P, :])

            e_s = rp.tile([P, L], dt.float32)
            r_s = rp.tile([P, L], dt.float32)
            em1 = rp.tile([P, L], dt.float32)
            cs = rp.tile([P, L], dt.float32)

            # ================= phase 1: y' = x@M^T+u ; d ; e ; r ==========
            for lb in range(NL):
                ls = slice(lb * LB, (lb + 1) * LB)
                xb = xp.tile([P, NH * LB], dt.float32r, tag="xb")
                nc.sync.dma_start(
                    xb[:].rearrange("p (j l) -> p j l", j=NH), xT3[:, :, ls])

                pd = dp.tile([P, LB], dt.float32)
                pending = None  # delayed d-matmul so PE never waits on DVE
                for ob in range(NH):
                    py = yp.tile([P, LB], dt.float32, tag="mm")
                    for hb in range(NH):
                        nc.tensor.matmul(
                            py[:],
                            mt[:, hb * H + ob * P: hb * H + ob * P + P],
                            xb[:, hb * LB:(hb + 1) * LB],
                            start=(hb == 0), stop=(hb == NH - 1))
                    yb = wp.tile([P, LB], dt.float32, tag="yb")
                    nc.scalar.activation(yb[:], py[:], AF.Identity,
                                         bias=ub[:, ob:ob + 1], scale=1.0)
                    prod = wp.tile([P, LB], dt.float32r, tag="prod")
                    nc.vector.tensor_tensor(
                        prod[:], yb[:],
                        xb[:, ob * LB:(ob + 1) * LB].bitcast(dt.float32),
                        op=ALU.mult)
                    if pending is not None:
                        pi, pprod = pending
                        nc.tensor.matmul(pd[:], ones[:], pprod[:],
                                         start=(pi == 0), stop=False)
                    pending = (ob, prod)
                pi, pprod = pending
                nc.tensor.matmul(pd[:], ones[:], pprod[:],
                                 start=(pi == 0), stop=True)

                nc.scalar.activation(e_s[:, ls], pd[:], AF.Exp,
                                     bias=c0b[:, 0:1], scale=1.0)
                tmp = wp.tile([P, LB], dt.float32, tag="tmp")
                nc.vector.tensor_scalar_add(tmp[:], e_s[:, ls], float(L - 1))
                nc.vector.reciprocal(r_s[:, ls], tmp[:])

            # ---- colsum = S + (e-1)*r  (tiny; overlaps phase-2 matmuls) ----
            S_t = rp.tile([P, 1], dt.float32)
            nc.vector.tensor_reduce(S_t[:], r_s[:], axis=mybir.AxisListType.X,
                                    op=ALU.add)
            nc.vector.tensor_scalar_sub(em1[:], e_s[:], 1.0)
            nc.vector.tensor_tensor(cs[:], em1[:], r_s[:], op=ALU.mult)
            nc.vector.tensor_scalar_add(cs[:], cs[:], S_t[:])

            # ================= phase 2: z = x@Wc^T+bc ; out ================
            for lb in range(NL):
                ls = slice(lb * LB, (lb + 1) * LB)
                xb = xp.tile([P, NH * LB], dt.float32r, tag="xb")
                nc.sync.dma_start(
                    xb[:].rearrange("p (j l) -> p j l", j=NH), xT3[:, :, ls])
                for ob in range(NH):
                    pz = yp.tile([P, LB], dt.float32, tag="mm")
                    for hb in range(NH):
                        nc.tensor.matmul(
                            pz[:],
                            wct[:, hb * H + ob * P: hb * H + ob * P + P],
                            xb[:, hb * LB:(hb + 1) * LB],
                            start=(hb == 0), stop=(hb == NH - 1))
                    zb = wp.tile([P, LB], dt.float32, tag="zb")
                    nc.scalar.activation(zb[:], pz[:], AF.Identity,
                                         bias=bcb[:, ob:ob + 1], scale=1.0)
                    zc = wp.tile([P, LB], dt.float32, tag="zc")
                    nc.vector.tensor_tensor(zc[:], zb[:], cs[:, ls], op=ALU.mult)
                    ot = wp.tile([P, LB], dt.float32, tag="ot")
                    nc.vector.tensor_scalar_add(ot[:], zc[:], bob[:, ob:ob + 1])
                    nc.sync.dma_start(out_d[ob * P:(ob + 1) * P, ls], ot[:])

    nc.compile()
    return nc


def _get_nc():
    if "nc" not in _CACHE:
        _CACHE["nc"] = _build()
    return _CACHE["nc"]


def _prep_inputs(x, Wq, bq, Wk, bk, Wv, bv, Wo, bo):
    """Host-side precompute (fp64 for the fused weights) + per-core sharding."""
    f8 = np.float64
    M = (Wq.astype(f8).T @ Wk.astype(f8)).astype(np.float32)
    u = (Wk.astype(f8).T @ bq.astype(f8)
         + Wq.astype(f8).T @ bk.astype(f8)).astype(np.float32)
    c0 = np.float32(bq.astype(f8) @ bk.astype(f8))
    Wc = (Wo.astype(f8) @ Wv.astype(f8)).astype(np.float32)
    bc = (Wo.astype(f8) @ bv.astype(f8)).astype(np.float32)

    MT = np.ascontiguousarray(M.T)
    WcT = np.ascontiguousarray(Wc.T)
    ub = np.ascontiguousarray(u.reshape(NH, P).T)
    bcb = np.ascontiguousarray(bc.reshape(NH, P).T)
    bob = np.ascontiguousarray(bo.astype(np.float32).reshape(NH, P).T)
    c0b = np.full((P, 1), c0, np.float32)
    ones = np.ones((P, P), np.float32)

    shared = dict(MT=MT, WcT=WcT, ub=ub, bcb=bcb, bob=bob, c0b=c0b, ones=ones)
    in_maps = []
    for n in range(N_CORES):
        xT = np.ascontiguousarray(x[n].astype(np.float32).T)
        in_maps.append(dict(xT=xT, **shared))
    return in_maps


def kernel(x, Wq, bq, Wk, bk, Wv, bv, Wo, bo, _trace=False, _trace_kwargs=None):
    nc = _get_nc()
    in_maps = _prep_inputs(x, Wq, bq, Wk, bk, Wv, bv, Wo, bo)
    res = run_bass_kernel_spmd(nc, in_maps, list(range(N_CORES)),
                               trace=_trace, **(_trace_kwargs or {}))
    out = np.empty((N, L, H), np.float32)
    for n in range(N_CORES):
        out[n] = res.results[n]["outT"].T
    if _trace:
        kernel.last_result = res
    return out


# revision 4
# speedup vs baseline: 1.1465x; 1.1465x over previous
"""TRN2 Bass kernel for nn_Encoder_27290222198965.

Reference computation (N=8, L=2048, H=1024):
    q = x@Wq.T+bq ; k = x@Wk.T+bk ; v = x@Wv.T+bv
    d[n,l] = sum_h q*k                       (diagonal "attention" scores)
    att = softmax(diag-embed(d), axis=2) ->  colsum[n,l] = S[n] + (e-1)/(L-1+e),
        e = exp(d[n,l]), S[n] = sum_l 1/(L-1+exp(d[n,l]))
    out = (colsum[:, :, None] * v) @ Wo.T + bo

Algebraic refactor (validated to ~4e-6 rel err with exact matmuls):
    d[n,l] = rowsum(x ⊙ y') + c0,  y' = x @ M^T + u,
        M = Wq^T Wk, u = Wk^T bq + Wq^T bk, c0 = bq·bk
    colsum = (S+1) - 2048*r,  r = 1/(2047+exp(d)),  S = sum_l r
        (uses e*r = 1 - 2047*r)
    out    = colsum ⊙ (x @ Wc^T + bc) + bo,  Wc = Wo@Wv, bc = Wo@bv
so only TWO HxH projections run on hardware (y' and z) instead of four.

Sharding: data-parallel over N — core n handles batch n. All matmuls in
float32r (full PE rate at free dim 512, ~e8m12 effective precision).
Everything on-chip is transposed ([feature, token]) so biases are
per-partition and fold into ScalarE psum->sbuf copies. DMA is emitted in
just-in-time consumption order (per-ob weight tiles, per-hb x tiles) so the
PE starts within ~2us of kernel start.
"""

import numpy as np

import concourse.bass as bass  # noqa: F401  (registers engines on Bacc)
import concourse.tile as tile
from concourse import bacc, mybir
from concourse.bass_utils import run_bass_kernel_spmd

dt = mybir.dt
AF = mybir.ActivationFunctionType
ALU = mybir.AluOpType

N, L, H = 8, 2048, 1024
P = 128            # SBUF partitions
LB = 512           # l-block (moving free dim of every matmul)
NH = H // P        # 8 h-blocks
NL = L // LB       # 4 l-blocks
N_CORES = 8

_CACHE = {}


def _build():
    nc = bacc.Bacc("TRN2", target_bir_lowering=False, debug=False,
                   num_devices=N_CORES)

    xT_d = nc.dram_tensor("xT", [H, L], dt.float32r, kind="ExternalInput").ap()
    MT_d = nc.dram_tensor("MT", [H, H], dt.float32r, kind="ExternalInput").ap()
    WcT_d = nc.dram_tensor("WcT", [H, H], dt.float32r, kind="ExternalInput").ap()
    ub_d = nc.dram_tensor("ub", [P, NH], dt.float32, kind="ExternalInput").ap()
    bcb_d = nc.dram_tensor("bcb", [P, NH], dt.float32, kind="ExternalInput").ap()
    bob_d = nc.dram_tensor("bob", [P, NH], dt.float32, kind="ExternalInput").ap()
    c0b_d = nc.dram_tensor("c0b", [P, 1], dt.float32, kind="ExternalInput").ap()
    ones_d = nc.dram_tensor("ones", [P, P], dt.float32r, kind="ExternalInput").ap()
    out_d = nc.dram_tensor("outT", [H, L], dt.float32, kind="ExternalOutput").ap()

    xT3 = xT_d.rearrange("(j p) l -> p j l", p=P)    # [128, 8, 2048]
    MT3 = MT_d.rearrange("(j p) c -> p j c", p=P)    # [128, 8, 1024]
    WcT3 = WcT_d.rearrange("(j p) c -> p j c", p=P)  # [128, 8, 1024]

    with tile.TileContext(nc) as tc:
        with (
            tc.tile_pool(name="resident", bufs=1) as rp,
            tc.tile_pool(name="weights", bufs=1) as wtp,
            tc.tile_pool(name="xstream", bufs=24) as xp,
            tc.tile_pool(name="work", bufs=3) as wp,
            tc.tile_pool(name="mmpsum", bufs=3, space="PSUM") as yp,
            tc.tile_pool(name="dpsum", bufs=2, space="PSUM") as dp,
        ):
            # tiny resident constants first (a few KB, negligible DMA)
            ub = rp.tile([P, NH], dt.float32)
            nc.sync.dma_start(ub[:], ub_d[:])
            c0b = rp.tile([P, 1], dt.float32)
            nc.sync.dma_start(c0b[:], c0b_d[:])
            ones = rp.tile([P, P], dt.float32r)
            nc.sync.dma_start(ones[:], ones_d[:])
            bcb = rp.tile([P, NH], dt.float32)
            nc.sync.dma_start(bcb[:], bcb_d[:])
            bob = rp.tile([P, NH], dt.float32)
            nc.sync.dma_start(bob[:], bob_d[:])

            r_s = rp.tile([P, L], dt.float32)
            cs = rp.tile([P, L], dt.float32)

            def load_w(src3, ob, tag):
                """One per-ob weight tile [hin(P), hb*P+hout] = 512KB."""
                t = wtp.tile([P, NH * P], dt.float32r, tag=f"{tag}{ob}")
                nc.sync.dma_start(
                    t[:].rearrange("p (j c) -> p j c", j=NH),
                    src3[:, :, ob * P:(ob + 1) * P])
                return t

            def load_xb(lb, hb):
                t = xp.tile([P, LB], dt.float32r, tag="xb")
                nc.sync.dma_start(t[:], xT3[:, hb, lb * LB:(lb + 1) * LB])
                return t

            # ---- JIT DMA emission for the cold start ----
            # interleave mt[ob0] consumption pair-wise with x block 0
            mt = [None] * NH
            wct_holder = [None] * NH
            xbs = {}
            mt[0] = load_w(MT3, 0, "mt")
            for hb in range(NH):
                xbs[(0, hb)] = load_xb(0, hb)
            for ob in range(1, NH):
                mt[ob] = load_w(MT3, ob, "mt")
            for hb in range(NH):
                xbs[(1, hb)] = load_xb(1, hb)

            # d-matmul bookkeeping: delay each block's last rowsum-MM into the
            # next MM group so the PE never waits on the ACT->DVE prod chain.
            state = {"pending": None}

            def flush_pending():
                if state["pending"] is None:
                    return
                pd_t, ob, prod_t, is_last, lb = state["pending"]
                nc.tensor.matmul(pd_t[:], ones[:], prod_t[:],
                                 start=(ob == 0), stop=is_last)
                state["pending"] = None
                if is_last:
                    # e = exp(d + c0); r = 1/(2047 + e)
                    ls = slice(lb * LB, (lb + 1) * LB)
                    et = wp.tile([P, LB], dt.float32, tag="et")
                    nc.scalar.activation(et[:], pd_t[:], AF.Exp,
                                         bias=c0b[:, 0:1], scale=1.0)
                    tmp = wp.tile([P, LB], dt.float32, tag="tmp")
                    nc.vector.tensor_scalar_add(tmp[:], et[:], float(L - 1))
                    nc.vector.reciprocal(r_s[:, ls], tmp[:])

            # ================= phase 1: y' ; d ; r ==================
            for lb in range(NL):
                pd = dp.tile([P, LB], dt.float32)
                for ob in range(NH):
                    py = yp.tile([P, LB], dt.float32, tag="mm")
                    for hb in range(NH):
                        nc.tensor.matmul(
                            py[:], mt[ob][:, hb * P:(hb + 1) * P],
                            xbs[(lb, hb)][:],
                            start=(hb == 0), stop=(hb == NH - 1))
                    flush_pending()
                    yb = wp.tile([P, LB], dt.float32, tag="yb")
                    nc.scalar.activation(yb[:], py[:], AF.Identity,
                                         bias=ub[:, ob:ob + 1], scale=1.0)
                    prod = wp.tile([P, LB], dt.float32r, tag="prod")
                    nc.vector.tensor_tensor(
                        prod[:], yb[:], xbs[(lb, ob)][:].bitcast(dt.float32),
                        op=ALU.mult)
                    state["pending"] = (pd, ob, prod, ob == NH - 1, lb)
                # prefetch x for block lb+2 of phase 1, or re-reads for phase 2
                nxt = lb + 2
                if nxt < NL:
                    for hb in range(NH):
                        xbs[(nxt, hb)] = load_xb(nxt, hb)
                elif nxt == NL:  # after block 2: phase-2 weights
                    for ob in range(NH):
                        wct_holder[ob] = load_w(WcT3, ob, "wct")
                else:            # after block 3: phase-2 x block 0
                    for hb in range(NH):
                        xbs[("p2", 0, hb)] = load_xb(0, hb)

            # ================= phase 2: z ; out ==================
            for lb in range(NL):
                for ob in range(NH):
                    pz = yp.tile([P, LB], dt.float32, tag="mm")
                    for hb in range(NH):
                        nc.tensor.matmul(
                            pz[:], wct_holder[ob][:, hb * P:(hb + 1) * P],
                            xbs[("p2", lb, hb)][:],
                            start=(hb == 0), stop=(hb == NH - 1))
                    if lb == 0 and ob == 0:
                        flush_pending()   # last d-MM of phase 1
                        # colsum = (S+1) - 2048*r
                        S_t = rp.tile([P, 1], dt.float32)
                        nc.vector.tensor_reduce(
                            S_t[:], r_s[:], axis=mybir.AxisListType.X,
                            op=ALU.add)
                        S1_t = rp.tile([P, 1], dt.float32)
                        nc.vector.tensor_scalar_add(S1_t[:], S_t[:], 1.0)
                        nc.vector.tensor_scalar(
                            cs[:], r_s[:], float(-(L)), S1_t[:],
                            op0=ALU.mult, op1=ALU.add)
                    ls = slice(lb * LB, (lb + 1) * LB)
                    zb = wp.tile([P, LB], dt.float32, tag="zb")
                    nc.scalar.activation(zb[:], pz[:], AF.Identity,
                                         bias=bcb[:, ob:ob + 1], scale=1.0)
                    zc = wp.tile([P, LB], dt.float32, tag="zc")
                    nc.vector.tensor_tensor(zc[:], zb[:], cs[:, ls],
                                            op=ALU.mult)
                    ot = wp.tile([P, LB], dt.float32, tag="ot")
                    nc.vector.tensor_scalar_add(ot[:], zc[:],
                                                bob[:, ob:ob + 1])
                    nc.sync.dma_start(out_d[ob * P:(ob + 1) * P, ls], ot[:])
                # prefetch phase-2 x for block lb+1
                if lb + 1 < NL:
                    for hb in range(NH):
                        xbs[("p2", lb + 1, hb)] = load_xb(lb + 1, hb)

    nc.compile()
    return nc


def _get_nc():
    if "nc" not in _CACHE:
        _CACHE["nc"] = _build()
    return _CACHE["nc"]


def _prep_inputs(x, Wq, bq, Wk, bk, Wv, bv, Wo, bo):
    """Host-side precompute (fp64 for the fused weights) + per-core sharding."""
    f8 = np.float64
    M = (Wq.astype(f8).T @ Wk.astype(f8)).astype(np.float32)
    u = (Wk.astype(f8).T @ bq.astype(f8)
         + Wq.astype(f8).T @ bk.astype(f8)).astype(np.float32)
    c0 = np.float32(bq.astype(f8) @ bk.astype(f8))
    Wc = (Wo.astype(f8) @ Wv.astype(f8)).astype(np.float32)
    bc = (Wo.astype(f8) @ bv.astype(f8)).astype(np.float32)

    MT = np.ascontiguousarray(M.T)
    WcT = np.ascontiguousarray(Wc.T)
    ub = np.ascontiguousarray(u.reshape(NH, P).T)
    bcb = np.ascontiguousarray(bc.reshape(NH, P).T)
    bob = np.ascontiguousarray(bo.astype(np.float32).reshape(NH, P).T)
    c0b = np.full((P, 1), c0, np.float32)
    ones = np.ones((P, P), np.float32)

    shared = dict(MT=MT, WcT=WcT, ub=ub, bcb=bcb, bob=bob, c0b=c0b, ones=ones)
    in_maps = []
    for n in range(N_CORES):
        xT = np.ascontiguousarray(x[n].astype(np.float32).T)
        in_maps.append(dict(xT=xT, **shared))
    return in_maps


def kernel(x, Wq, bq, Wk, bk, Wv, bv, Wo, bo, _trace=False, _trace_kwargs=None):
    nc = _get_nc()
    in_maps = _prep_inputs(x, Wq, bq, Wk, bk, Wv, bv, Wo, bo)
    res = run_bass_kernel_spmd(nc, in_maps, list(range(N_CORES)),
                               trace=_trace, **(_trace_kwargs or {}))
    out = np.empty((N, L, H), np.float32)
    for n in range(N_CORES):
        out[n] = res.results[n]["outT"].T
    if _trace:
        kernel.last_result = res
    return out


# revision 28
# speedup vs baseline: 1.3322x; 1.1620x over previous
"""TRN2 Bass kernel for nn_Encoder_27290222198965.

Reference computation (N=8, L=2048, H=1024):
    q = x@Wq.T+bq ; k = x@Wk.T+bk ; v = x@Wv.T+bv
    d[n,l] = sum_h q*k                       (diagonal "attention" scores)
    att = softmax(diag-embed(d), axis=2) ->  colsum[n,l] = S[n] + (e-1)/(L-1+e),
        e = exp(d[n,l]), S[n] = sum_l 1/(L-1+exp(d[n,l]))
    out = (colsum[:, :, None] * v) @ Wo.T + bo

Algebraic refactor (validated to ~4e-6 rel err with exact matmuls):
    d[n,l] = rowsum(x ⊙ y') + c0,  y' = x @ M^T + u,
        M = Wq^T Wk, u = Wk^T bq + Wq^T bk, c0 = bq·bk
    colsum = (S+1) - 2048*r,  r = 1/(2047+exp(d)),  S = sum_l r
        (uses e*r = 1 - 2047*r)
    out    = colsum ⊙ (x @ Wc^T + bc) + bo,  Wc = Wo@Wv, bc = Wo@bv
so only TWO HxH projections run on hardware (y' and z) instead of four.

Sharding: data-parallel over N — core n handles batch n. All matmuls in
float32r (full PE rate at free dim 512, ~e8m12 effective precision).
Everything on-chip is transposed ([feature, token]) so biases are
per-partition and fold into ScalarE psum->sbuf copies. DMA is emitted in
just-in-time consumption order (per-ob weight tiles, per-hb x tiles) so the
PE starts within ~2us of kernel start.
"""

import numpy as np

import concourse.bass as bass  # noqa: F401  (registers engines on Bacc)
import concourse.tile as tile
from concourse import bacc, mybir
from concourse.bass_utils import run_bass_kernel_spmd

dt = mybir.dt
AF = mybir.ActivationFunctionType
ALU = mybir.AluOpType

N, L, H = 8, 2048, 1024
P = 128            # SBUF partitions
LB = 512           # l-block (moving free dim of every matmul)
NH = H // P        # 8 h-blocks
NL = L // LB       # 4 l-blocks
N_CORES = 8

_CACHE = {}


def _build():
    nc = bacc.Bacc("TRN2", target_bir_lowering=False, debug=False,
                   num_devices=N_CORES)

    xT_d = nc.dram_tensor("xT", [H, L], dt.float32r, kind="ExternalInput").ap()
    MT_d = nc.dram_tensor("MT", [NH, P, NH * P], dt.float32r, kind="ExternalInput").ap()
    WcT_d = nc.dram_tensor("WcT", [NH, P, NH * P], dt.float32r, kind="ExternalInput").ap()
    cp_d = nc.dram_tensor("cpack", [P, NH + 1 + P], dt.float32r,
                          kind="ExternalInput").ap()
    bcb_d = nc.dram_tensor("bcb", [P, NH], dt.float32, kind="ExternalInput").ap()
    bob_d = nc.dram_tensor("bob", [P, NH], dt.float32, kind="ExternalInput").ap()
    out_d = nc.dram_tensor("outT", [H, L], dt.float32, kind="ExternalOutput").ap()

    xT3 = xT_d.rearrange("(j p) l -> p j l", p=P)    # [128, 8, 2048]
    MT3 = MT_d    # prepacked [ob, p(hin%128), hb*128+hout]
    WcT3 = WcT_d

    with tile.TileContext(nc) as tc:
        with (
            tc.tile_pool(name="resident", bufs=1) as rp,
            tc.tile_pool(name="weights", bufs=1) as wtp,
            tc.tile_pool(name="xstream", bufs=24) as xp,
            tc.tile_pool(name="work", bufs=3) as wp,
            tc.tile_pool(name="mmpsum", bufs=4, space="PSUM") as yp,
            tc.tile_pool(name="dpsum", bufs=2, space="PSUM") as dp,
        ):
            t_s = rp.tile([P, L], dt.float32)
            cs = rp.tile([P, L], dt.float32)

            def load_w(src3, ob, tag, eng=None):
                """One per-ob weight tile [hin(P), hb*P+hout] = 512KB."""
                t = wtp.tile([P, NH * P], dt.float32r, tag=f"{tag}{ob}")
                (eng or nc.sync).dma_start(t[:], src3[ob])
                return t

            def load_xb(lb, hb, eng=None):
                t = xp.tile([P, LB], dt.float32r, tag="xb")
                (eng or nc.sync).dma_start(
                    t[:], xT3[:, hb, lb * LB:(lb + 1) * LB])
                return t

            # ---- JIT DMA emission for the cold start ----
            mt = [None] * NH
            wct_holder = [None] * NH
            consts = {}
            xbs = {}
            cp = rp.tile([P, NH + 1 + P], dt.float32r)
            nc.sync.dma_start(cp[:], cp_d[:])
            ub = cp[:, :NH].bitcast(dt.float32)
            c0b = cp[:, NH:NH + 1].bitcast(dt.float32)
            ones = cp[:, NH + 1:]
            mt[0] = load_w(MT3, 0, "mt", eng=nc.scalar)
            for hb in range(NH):
                xbs[(0, hb)] = load_xb(0, hb)
            for ob in range(1, NH):
                mt[ob] = load_w(MT3, ob, "mt", eng=nc.scalar)
            for hb in range(NH):
                xbs[(1, hb)] = load_xb(1, hb)

            # d-matmul bookkeeping: delay each block's last rowsum-MM into the
            # next MM group so the PE never waits on the ACT->DVE prod chain.
            state = {"pending": None}

            def flush_pending():
                if state["pending"] is None:
                    return
                pd_t, ob, prod_t, is_last, lb = state["pending"]
                nc.tensor.matmul(pd_t[:], ones, prod_t[:],
                                 start=(ob == 0), stop=is_last)
                state["pending"] = None
                if is_last:
                    # t = sigmoid(-d - c0 + ln(L-1)); r = t/(L-1)
                    # (1/((L-1)+e^d) = sigmoid(-d+ln(L-1))/(L-1))
                    ls = slice(lb * LB, (lb + 1) * LB)
                    nc.scalar.activation(t_s[:, ls], pd_t[:], AF.Sigmoid,
                                         bias=c0b[:, 0:1], scale=-1.0)

            # ================= phase 1: y' ; d ; r ==================
            for lb in range(NL):
                pd = dp.tile([P, LB], dt.float32)
                acc = None
                for ob in range(NH):
                    py = yp.tile([P, LB], dt.float32, tag="mm")
                    for hb in range(NH):
                        nc.tensor.matmul(
                            py[:], mt[ob][:, hb * P:(hb + 1) * P],
                            xbs[(lb, hb)][:],
                            start=(hb == 0), stop=(hb == NH - 1))
                    if ob == 1:
                        flush_pending()
                    yb = wp.tile([P, LB], dt.float32, tag="yb")
                    nc.scalar.activation(yb[:], py[:], AF.Identity,
                                         bias=ub[:, ob:ob + 1], scale=1.0)
                    prod = wp.tile([P, LB], dt.float32r, tag="prod")
                    nc.vector.tensor_tensor(
                        prod[:], yb[:], xbs[(lb, ob)][:].bitcast(dt.float32),
                        op=ALU.mult)
                    if acc is None:
                        acc = prod
                    else:
                        nacc = wp.tile([P, LB], dt.float32r, tag="pacc")
                        nc.vector.tensor_tensor(nacc[:], acc[:], prod[:],
                                                op=ALU.add)
                        acc = nacc
                state["pending"] = (pd, 0, acc, True, lb)
                # prefetch x for block lb+2 of phase 1, or re-reads for phase 2
                nxt = lb + 2
                if nxt < NL:
                    for hb in range(NH):
                        xbs[(nxt, hb)] = load_xb(nxt, hb)
                elif nxt == NL:  # after block 2: phase-2 weights
                    consts["bcb"] = rp.tile([P, NH], dt.float32, name="bcbt", tag="bcb")
                    nc.sync.dma_start(consts["bcb"][:], bcb_d[:])
                    consts["bob"] = rp.tile([P, NH], dt.float32, name="bobt", tag="bob")
                    nc.sync.dma_start(consts["bob"][:], bob_d[:])
                    for ob in range(NH):
                        wct_holder[ob] = load_w(WcT3, ob, "wct")
                else:            # after block 3: phase-2 x block 0
                    for hb in range(NH):
                        xbs[("p2", 0, hb)] = load_xb(0, hb)

            # ================= phase 2: z ; out ==================
            for lb in range(NL):
                for ob in range(NH):
                    pz = yp.tile([P, LB], dt.float32, tag="mm")
                    for hb in range(NH):
                        nc.tensor.matmul(
                            pz[:], wct_holder[ob][:, hb * P:(hb + 1) * P],
                            xbs[("p2", lb, hb)][:],
                            start=(hb == 0), stop=(hb == NH - 1))
                    if lb == 0 and ob == 0:
                        flush_pending()   # last d-MM of phase 1
                        # colsum = (1 + sum(t)/(L-1)) - (L/(L-1))*t
                        S_t = rp.tile([P, 1], dt.float32)
                        nc.vector.tensor_reduce(
                            S_t[:], t_s[:], axis=mybir.AxisListType.X,
                            op=ALU.add)
                        S1_t = rp.tile([P, 1], dt.float32)
                        nc.vector.tensor_scalar(
                            S1_t[:], S_t[:], 1.0 / (L - 1), 1.0,
                            op0=ALU.mult, op1=ALU.add)
                        nc.vector.tensor_scalar(
                            cs[:], t_s[:], -float(L) / (L - 1), S1_t[:],
                            op0=ALU.mult, op1=ALU.add)
                    # final group: half-width chunks so the last out-DMA
                    # starts ~1.3us earlier (shorter kernel tail)
                    nchunk = 2 if (lb == NL - 1 and ob == NH - 1) else 1
                    cw = LB // nchunk
                    for ck in range(nchunk):
                        lo = lb * LB + ck * cw
                        lsc = slice(lo, lo + cw)
                        pzc = pz[:, ck * cw:(ck + 1) * cw]
                        zb = wp.tile([P, cw], dt.float32, tag="zb")
                        nc.scalar.activation(zb[:], pzc,
                                             AF.Identity,
                                             bias=consts["bcb"][:, ob:ob + 1],
                                             scale=1.0)
                        zc = wp.tile([P, cw], dt.float32, tag="zc")
                        nc.vector.tensor_tensor(zc[:], zb[:], cs[:, lsc],
                                                op=ALU.mult)
                        ot = wp.tile([P, cw], dt.float32, tag="ot")
                        nc.vector.tensor_scalar_add(
                            ot[:], zc[:], consts["bob"][:, ob:ob + 1])
                        nc.sync.dma_start(
                            out_d[ob * P:(ob + 1) * P, lsc], ot[:])
                # prefetch phase-2 x for block lb+1
                if lb + 1 < NL:
                    for hb in range(NH):
                        xbs[("p2", lb + 1, hb)] = load_xb(lb + 1, hb)

    nc.compile()
    return nc


def _get_nc():
    if "nc" not in _CACHE:
        _CACHE["nc"] = _build()
    return _CACHE["nc"]


def _prep_inputs(x, Wq, bq, Wk, bk, Wv, bv, Wo, bo):
    """Host-side precompute (fp64 for the fused weights) + per-core sharding."""
    f8 = np.float64
    M = (Wq.astype(f8).T @ Wk.astype(f8)).astype(np.float32)
    u = (Wk.astype(f8).T @ bq.astype(f8)
         + Wq.astype(f8).T @ bk.astype(f8)).astype(np.float32)
    c0 = np.float32(bq.astype(f8) @ bk.astype(f8))
    Wc = (Wo.astype(f8) @ Wv.astype(f8)).astype(np.float32)
    bc = (Wo.astype(f8) @ bv.astype(f8)).astype(np.float32)

    def _pack(WT):  # [H,H] (hin, hout) -> [NH(ob), P(hin%P), NH*P]
        t = WT.reshape(NH, P, NH, P)          # [hb, p, ob, c]
        return np.ascontiguousarray(t.transpose(2, 1, 0, 3).reshape(NH, P, NH * P))

    MT = _pack(M.T)
    WcT = _pack(Wc.T)
    ub = np.ascontiguousarray(u.reshape(NH, P).T)
    bcb = np.ascontiguousarray(bc.reshape(NH, P).T)
    bob = np.ascontiguousarray(bo.astype(np.float32).reshape(NH, P).T)
    c0b = np.full((P, 1), np.log(L - 1.0) - np.float64(c0), np.float32)
    ones = np.ones((P, P), np.float32)
    cpack = np.concatenate([ub, c0b, ones], axis=1)

    shared = dict(MT=MT, WcT=WcT, cpack=cpack, bcb=bcb, bob=bob)
    in_maps = []
    for n in range(N_CORES):
        xT = np.ascontiguousarray(x[n].astype(np.float32).T)
        in_maps.append(dict(xT=xT, **shared))
    return in_maps


def kernel(x, Wq, bq, Wk, bk, Wv, bv, Wo, bo, _trace=False, _trace_kwargs=None):
    x, Wq, bq, Wk, bk, Wv, bv, Wo, bo = (
        np.asarray(a) for a in (x, Wq, bq, Wk, bk, Wv, bv, Wo, bo))
    nc = _get_nc()
    in_maps = _prep_inputs(x, Wq, bq, Wk, bk, Wv, bv, Wo, bo)
    res = run_bass_kernel_spmd(nc, in_maps, list(range(N_CORES)),
                               trace=_trace, **(_trace_kwargs or {}))
    out = np.empty((N, L, H), np.float32)
    for n in range(N_CORES):
        out[n] = res.results[n]["outT"].T
    if _trace:
        kernel.last_result = res
    return out


# revision 29
# speedup vs baseline: 1.3480x; 1.0119x over previous
"""TRN2 Bass kernel for nn_Encoder_27290222198965.

Reference computation (N=8, L=2048, H=1024):
    q = x@Wq.T+bq ; k = x@Wk.T+bk ; v = x@Wv.T+bv
    d[n,l] = sum_h q*k                       (diagonal "attention" scores)
    att = softmax(diag-embed(d), axis=2) ->  colsum[n,l] = S[n] + (e-1)/(L-1+e),
        e = exp(d[n,l]), S[n] = sum_l 1/(L-1+exp(d[n,l]))
    out = (colsum[:, :, None] * v) @ Wo.T + bo

Algebraic refactor (validated to ~4e-6 rel err with exact matmuls):
    d[n,l] = rowsum(x ⊙ y') + c0,  y' = x @ M^T + u,
        M = Wq^T Wk, u = Wk^T bq + Wq^T bk, c0 = bq·bk
    colsum = (S+1) - 2048*r,  r = 1/(2047+exp(d)),  S = sum_l r
        (uses e*r = 1 - 2047*r)
    out    = colsum ⊙ (x @ Wc^T + bc) + bo,  Wc = Wo@Wv, bc = Wo@bv
so only TWO HxH projections run on hardware (y' and z) instead of four.

Sharding: data-parallel over N — core n handles batch n. All matmuls in
float32r (full PE rate at free dim 512, ~e8m12 effective precision).
Everything on-chip is transposed ([feature, token]) so biases are
per-partition and fold into ScalarE psum->sbuf copies. DMA is emitted in
just-in-time consumption order (per-ob weight tiles, per-hb x tiles) so the
PE starts within ~2us of kernel start.
"""

import numpy as np

import concourse.bass as bass  # noqa: F401  (registers engines on Bacc)
import concourse.tile as tile
from concourse import bacc, mybir
from concourse.bass_utils import run_bass_kernel_spmd

dt = mybir.dt
AF = mybir.ActivationFunctionType
ALU = mybir.AluOpType

N, L, H = 8, 2048, 1024
P = 128            # SBUF partitions
LB = 512           # l-block (moving free dim of every matmul)
NH = H // P        # 8 h-blocks
NL = L // LB       # 4 l-blocks
N_CORES = 8

_CACHE = {}


def _build():
    nc = bacc.Bacc("TRN2", target_bir_lowering=False, debug=False,
                   num_devices=N_CORES)

    xT_d = nc.dram_tensor("xT", [H, L], dt.float32r, kind="ExternalInput").ap()
    MT_d = nc.dram_tensor("MT", [NH, P, NH * P], dt.float32r, kind="ExternalInput").ap()
    WcT_d = nc.dram_tensor("WcT", [NH, P, NH * P], dt.float32r, kind="ExternalInput").ap()
    cp_d = nc.dram_tensor("cpack", [P, NH + 1 + P], dt.float32r,
                          kind="ExternalInput").ap()
    bcb_d = nc.dram_tensor("bcb", [P, NH], dt.float32, kind="ExternalInput").ap()
    bob_d = nc.dram_tensor("bob", [P, NH], dt.float32, kind="ExternalInput").ap()
    out_d = nc.dram_tensor("outT", [H, L], dt.float32, kind="ExternalOutput").ap()

    xT3 = xT_d.rearrange("(j p) l -> p j l", p=P)    # [128, 8, 2048]
    MT3 = MT_d    # prepacked [ob, p(hin%128), hb*128+hout]
    WcT3 = WcT_d

    with tile.TileContext(nc) as tc:
        with (
            tc.tile_pool(name="resident", bufs=1) as rp,
            tc.tile_pool(name="weights", bufs=1) as wtp,
            tc.tile_pool(name="xstream", bufs=24) as xp,
            tc.tile_pool(name="work", bufs=3) as wp,
            tc.tile_pool(name="mmpsum", bufs=4, space="PSUM") as yp,
            tc.tile_pool(name="dpsum", bufs=2, space="PSUM") as dp,
        ):
            t_s = rp.tile([P, L], dt.float32)
            cs = rp.tile([P, L], dt.float32)

            def load_w(src3, ob, tag, eng=None):
                """One per-ob weight tile [hin(P), hb*P+hout] = 512KB."""
                t = wtp.tile([P, NH * P], dt.float32r, tag=f"{tag}{ob}")
                (eng or nc.sync).dma_start(t[:], src3[ob])
                return t

            def load_xb(lb, hb, eng=None):
                t = xp.tile([P, LB], dt.float32r, tag="xb")
                (eng or nc.sync).dma_start(
                    t[:], xT3[:, hb, lb * LB:(lb + 1) * LB])
                return t

            # ---- JIT DMA emission for the cold start ----
            mt = [None] * NH
            wct_holder = [None] * NH
            consts = {}
            xbs = {}
            cp = rp.tile([P, NH + 1 + P], dt.float32r)
            mt[0] = load_w(MT3, 0, "mt", eng=nc.scalar)
            for hb in range(NH):
                xbs[(0, hb)] = load_xb(0, hb)
                if hb == 3:  # consts mid-stream: needed only from t~15us on
                    nc.sync.dma_start(cp[:], cp_d[:])
            ub = cp[:, :NH].bitcast(dt.float32)
            c0b = cp[:, NH:NH + 1].bitcast(dt.float32)
            ones = cp[:, NH + 1:]
            for ob in range(1, NH):
                mt[ob] = load_w(MT3, ob, "mt", eng=nc.scalar)
            for hb in range(NH):
                xbs[(1, hb)] = load_xb(1, hb)

            # d-matmul bookkeeping: delay each block's last rowsum-MM into the
            # next MM group so the PE never waits on the ACT->DVE prod chain.
            state = {"pending": None}

            def flush_pending():
                if state["pending"] is None:
                    return
                pd_t, ob, prod_t, is_last, lb = state["pending"]
                nc.tensor.matmul(pd_t[:], ones, prod_t[:],
                                 start=(ob == 0), stop=is_last)
                state["pending"] = None
                if is_last:
                    # t = sigmoid(-d - c0 + ln(L-1)); r = t/(L-1)
                    # (1/((L-1)+e^d) = sigmoid(-d+ln(L-1))/(L-1))
                    ls = slice(lb * LB, (lb + 1) * LB)
                    nc.scalar.activation(t_s[:, ls], pd_t[:], AF.Sigmoid,
                                         bias=c0b[:, 0:1], scale=-1.0)

            # ================= phase 1: y' ; d ; r ==================
            for lb in range(NL):
                pd = dp.tile([P, LB], dt.float32)
                acc = None
                for ob in range(NH):
                    py = yp.tile([P, LB], dt.float32, tag="mm")
                    for hb in range(NH):
                        nc.tensor.matmul(
                            py[:], mt[ob][:, hb * P:(hb + 1) * P],
                            xbs[(lb, hb)][:],
                            start=(hb == 0), stop=(hb == NH - 1))
                    if ob == 1:
                        flush_pending()
                    yb = wp.tile([P, LB], dt.float32, tag="yb")
                    nc.scalar.activation(yb[:], py[:], AF.Identity,
                                         bias=ub[:, ob:ob + 1], scale=1.0)
                    prod = wp.tile([P, LB], dt.float32r, tag="prod")
                    nc.vector.tensor_tensor(
                        prod[:], yb[:], xbs[(lb, ob)][:].bitcast(dt.float32),
                        op=ALU.mult)
                    if acc is None:
                        acc = prod
                    else:
                        nacc = wp.tile([P, LB], dt.float32r, tag="pacc")
                        nc.vector.tensor_tensor(nacc[:], acc[:], prod[:],
                                                op=ALU.add)
                        acc = nacc
                state["pending"] = (pd, 0, acc, True, lb)
                # prefetch x for block lb+2 of phase 1, or re-reads for phase 2
                nxt = lb + 2
                if nxt < NL:
                    for hb in range(NH):
                        xbs[(nxt, hb)] = load_xb(nxt, hb)
                elif nxt == NL:  # after block 2: phase-2 weights
                    consts["bcb"] = rp.tile([P, NH], dt.float32, name="bcbt", tag="bcb")
                    nc.sync.dma_start(consts["bcb"][:], bcb_d[:])
                    consts["bob"] = rp.tile([P, NH], dt.float32, name="bobt", tag="bob")
                    nc.sync.dma_start(consts["bob"][:], bob_d[:])
                    for ob in range(NH):
                        wct_holder[ob] = load_w(WcT3, ob, "wct")
                else:            # after block 3: phase-2 x block 0
                    for hb in range(NH):
                        xbs[("p2", 0, hb)] = load_xb(0, hb)

            # ================= phase 2: z ; out ==================
            for lb in range(NL):
                for ob in range(NH):
                    pz = yp.tile([P, LB], dt.float32, tag="mm")
                    for hb in range(NH):
                        nc.tensor.matmul(
                            pz[:], wct_holder[ob][:, hb * P:(hb + 1) * P],
                            xbs[("p2", lb, hb)][:],
                            start=(hb == 0), stop=(hb == NH - 1))
                    if lb == 0 and ob == 0:
                        flush_pending()   # last d-MM of phase 1
                        # colsum = (1 + sum(t)/(L-1)) - (L/(L-1))*t
                        S_t = rp.tile([P, 1], dt.float32)
                        nc.vector.tensor_reduce(
                            S_t[:], t_s[:], axis=mybir.AxisListType.X,
                            op=ALU.add)
                        S1_t = rp.tile([P, 1], dt.float32)
                        nc.vector.tensor_scalar(
                            S1_t[:], S_t[:], 1.0 / (L - 1), 1.0,
                            op0=ALU.mult, op1=ALU.add)
                        nc.vector.tensor_scalar(
                            cs[:], t_s[:], -float(L) / (L - 1), S1_t[:],
                            op0=ALU.mult, op1=ALU.add)
                    # final group: half-width chunks so the last out-DMA
                    # starts ~1.3us earlier (shorter kernel tail)
                    nchunk = 2 if (lb == NL - 1 and ob == NH - 1) else 1
                    cw = LB // nchunk
                    for ck in range(nchunk):
                        lo = lb * LB + ck * cw
                        lsc = slice(lo, lo + cw)
                        pzc = pz[:, ck * cw:(ck + 1) * cw]
                        zb = wp.tile([P, cw], dt.float32, tag="zb")
                        nc.scalar.activation(zb[:], pzc,
                                             AF.Identity,
                                             bias=consts["bcb"][:, ob:ob + 1],
                                             scale=1.0)
                        zc = wp.tile([P, cw], dt.float32, tag="zc")
                        nc.vector.tensor_tensor(zc[:], zb[:], cs[:, lsc],
                                                op=ALU.mult)
                        ot = wp.tile([P, cw], dt.float32, tag="ot")
                        nc.vector.tensor_scalar_add(
                            ot[:], zc[:], consts["bob"][:, ob:ob + 1])
                        nc.sync.dma_start(
                            out_d[ob * P:(ob + 1) * P, lsc], ot[:])
                # prefetch phase-2 x for block lb+1
                if lb + 1 < NL:
                    for hb in range(NH):
                        xbs[("p2", lb + 1, hb)] = load_xb(lb + 1, hb)

    nc.compile()
    return nc


def _get_nc():
    if "nc" not in _CACHE:
        _CACHE["nc"] = _build()
    return _CACHE["nc"]


def _prep_inputs(x, Wq, bq, Wk, bk, Wv, bv, Wo, bo):
    """Host-side precompute (fp64 for the fused weights) + per-core sharding."""
    f8 = np.float64
    M = (Wq.astype(f8).T @ Wk.astype(f8)).astype(np.float32)
    u = (Wk.astype(f8).T @ bq.astype(f8)
         + Wq.astype(f8).T @ bk.astype(f8)).astype(np.float32)
    c0 = np.float32(bq.astype(f8) @ bk.astype(f8))
    Wc = (Wo.astype(f8) @ Wv.astype(f8)).astype(np.float32)
    bc = (Wo.astype(f8) @ bv.astype(f8)).astype(np.float32)

    def _pack(WT):  # [H,H] (hin, hout) -> [NH(ob), P(hin%P), NH*P]
        t = WT.reshape(NH, P, NH, P)          # [hb, p, ob, c]
        return np.ascontiguousarray(t.transpose(2, 1, 0, 3).reshape(NH, P, NH * P))

    MT = _pack(M.T)
    WcT = _pack(Wc.T)
    ub = np.ascontiguousarray(u.reshape(NH, P).T)
    bcb = np.ascontiguousarray(bc.reshape(NH, P).T)
    bob = np.ascontiguousarray(bo.astype(np.float32).reshape(NH, P).T)
    c0b = np.full((P, 1), np.log(L - 1.0) - np.float64(c0), np.float32)
    ones = np.ones((P, P), np.float32)
    cpack = np.concatenate([ub, c0b, ones], axis=1)

    shared = dict(MT=MT, WcT=WcT, cpack=cpack, bcb=bcb, bob=bob)
    in_maps = []
    for n in range(N_CORES):
        xT = np.ascontiguousarray(x[n].astype(np.float32).T)
        in_maps.append(dict(xT=xT, **shared))
    return in_maps


def kernel(x, Wq, bq, Wk, bk, Wv, bv, Wo, bo, _trace=False, _trace_kwargs=None):
    x, Wq, bq, Wk, bk, Wv, bv, Wo, bo = (
        np.asarray(a) for a in (x, Wq, bq, Wk, bk, Wv, bv, Wo, bo))
    nc = _get_nc()
    in_maps = _prep_inputs(x, Wq, bq, Wk, bk, Wv, bv, Wo, bo)
    res = run_bass_kernel_spmd(nc, in_maps, list(range(N_CORES)),
                               trace=_trace, **(_trace_kwargs or {}))
    out = np.empty((N, L, H), np.float32)
    for n in range(N_CORES):
        out[n] = res.results[n]["outT"].T
    if _trace:
        kernel.last_result = res
    return out
